# revision 25
# baseline (speedup 1.0000x reference)
# Multi-head attention block (QKV proj + per-head q/k layernorm + softmax
# attention + output proj) on 8 Trainium2 NeuronCores.
#
# Sharding: data-parallel over (batch, query-half). Core c handles batch
# c//2, query tokens [ (c%2)*1024, (c%2+1)*1024 ). Each core computes K/V
# for its batch's full 2048 tokens; no cross-core communication, the host
# concatenates the 8 disjoint output chunks.
#
# On-device dataflow per core:
#   q,k are produced directly in [feature, token] layout (stationary =
#   Wqkv block, moving = xT) so no DMA transposes are needed; the qkv bias
#   rides the PSUM eviction as a per-partition ACT bias. LayerNorm stats
#   (per-head mean/rstd over D=64 = partition groups) come from ones-block
#   matmuls on the PE (ones scaled by 1/64 so PSUM holds the means); mu and
#   rstd are broadcast back over partitions with a DRAM-bounce DMA and
#   applied with two vector ops. q is stored zero-padded per head
#   ([q_head;0] / [0;q_head]) so score matmuls contract over K=128 and four
#   moving streams share one stationary load. v is natural-layout with a
#   ones column so the softmax normalizer Z rides attn@v as PSUM row 64.
#   The v bias is folded into beff = bv @ Wproj + bproj on the host
#   (softmax rows sum to one, so this is exact).
import contextlib

import numpy as np
import ml_dtypes

B, T, E = 4, 2048, 1024
H, D = 16, 64
P = 128
EPS = 1e-5
SCALE = 0.125  # 1/sqrt(D)
TQ = T // 2          # query tokens per core
KB = E // P          # contraction blocks
FC = E // P          # feature chunks for q/k (2 heads each)
MKV = T // P         # kv token tiles
NCORES = 8

_BUILT = {}
_last_in_maps = None
DEBUG = False


def _build_real(affine: bool):
    import concourse.bass as bass
    import concourse.bacc as bacc
    import concourse.tile as tile
    from concourse import mybir

    f32 = mybir.dt.float32
    bf16 = mybir.dt.bfloat16
    AF = mybir.ActivationFunctionType
    OP = mybir.AluOpType

    nc = bacc.Bacc("TRN2", target_bir_lowering=False)
    xT_q = nc.declare_dram_parameter("xT_q", [E, TQ], bf16, isOutput=False)
    xT_kv = nc.declare_dram_parameter("xT_kv", [E, T], bf16, isOutput=False)
    Wqkv = nc.declare_dram_parameter("Wqkv", [E, 3 * E], bf16, isOutput=False)
    bqkv = nc.declare_dram_parameter("bqkv", [3 * E], f32, isOutput=False)
    beff = nc.declare_dram_parameter("beff", [E], bf16, isOutput=False)
    if affine:
        q_gamma = nc.declare_dram_parameter("q_gamma", [D], f32, isOutput=False)
        q_beta = nc.declare_dram_parameter("q_beta", [D], f32, isOutput=False)
        k_gamma = nc.declare_dram_parameter("k_gamma", [D], f32, isOutput=False)
        k_beta = nc.declare_dram_parameter("k_beta", [D], f32, isOutput=False)
    Wproj = nc.declare_dram_parameter("Wproj", [E, E], bf16, isOutput=False)
    out = nc.declare_dram_parameter("out", [TQ, E], f32, isOutput=True)
    if DEBUG:
        dbg_q = nc.declare_dram_parameter("dbg_q", [P, 2, FC, TQ], bf16, isOutput=True)
        dbg_k = nc.declare_dram_parameter("dbg_k", [P, FC, T], bf16, isOutput=True)
        dbg_va = nc.declare_dram_parameter("dbg_va", [P, MKV, H, D + 1], bf16, isOutput=True)
        dbg_y = nc.declare_dram_parameter("dbg_y", [P, FC, TQ], bf16, isOutput=True)

    def bc_read(dst, tensor_ap, elem_off, reps, inner_ap):
        # broadcast-read: dst[p, ...] = src[elem_off + inner] for all p
        ap = bass.AP(tensor=tensor_ap.tensor,
                     offset=tensor_ap.offset + elem_off,
                     ap=[[0, reps], *inner_ap])
        nc.gpsimd.dma_start(out=dst, in_=ap)

    with tile.TileContext(nc) as tc, contextlib.ExitStack() as top:
        const = top.enter_context(tc.tile_pool(name="const", bufs=1))
        persist = top.enter_context(tc.tile_pool(name="persist", bufs=1))
        dr = top.enter_context(tc.tile_pool(name="dr", bufs=1, space="DRAM"))

        ones1 = const.tile([1, P], bf16)
        nc.vector.memset(ones1[:], 1.0)
        ones_bd = const.tile([P, 2], bf16)
        nc.vector.memset(ones_bd[:], 0.0)
        nc.vector.memset(ones_bd[0:64, 0:1], 1.0 / 64.0)
        nc.vector.memset(ones_bd[64:128, 1:2], 1.0 / 64.0)
        eps2 = const.tile([2, 1], f32)
        nc.vector.memset(eps2[:], EPS)
        bcol = const.tile([P, 16], f32)   # q/k bias, per-partition columns
        nc.sync.dma_start(out=bcol[:],
                          in_=bqkv[0:2 * E].rearrange("(c p) -> p c", p=P))
        beff_row = const.tile([1, E], bf16)
        nc.sync.dma_start(out=beff_row[:], in_=beff[:])
        if affine:
            gq_c = const.tile([P, 1], f32)
            bq_c = const.tile([P, 1], f32)
            gk_c = const.tile([P, 1], f32)
            bk_c = const.tile([P, 1], f32)
            for cc, src in ((gq_c, q_gamma), (bq_c, q_beta),
                            (gk_c, k_gamma), (bk_c, k_beta)):
                nc.sync.dma_start(out=cc[0:64, :], in_=src[:])
                nc.sync.dma_start(out=cc[64:128, :], in_=src[:])

        # qhat[:, 0] = [q_even; 0], qhat[:, 1] = [0; q_odd] (K=128 scores)
        qhat = persist.tile([P, 2, FC, TQ], bf16)
        nc.vector.memset(qhat[64:128, 0, :, :], 0.0)
        nc.vector.memset(qhat[0:64, 1, :, :], 0.0)
        khat = persist.tile([P, FC, T], bf16)
        # v + ones column (softmax normalizer Z rides as row 64 of attn@v)
        va = persist.tile([P, MKV, H, D + 1], bf16)
        nc.vector.memset(va[:, :, :, D], 1.0)
        yT = persist.tile([P, FC, TQ], bf16)

        # ---- phase A: projections + layernorm ----
        with contextlib.ExitStack() as pa:
            xkpool = pa.enter_context(tc.tile_pool(name="xkpool", bufs=1))
            wpool = pa.enter_context(tc.tile_pool(name="wpool", bufs=2))
            work = pa.enter_context(tc.tile_pool(name="work", bufs=1))
            ps = pa.enter_context(tc.tile_pool(name="psA", bufs=1, space="PSUM"))

            def ln_post(raw, S, Q2, dsts, gc, bc):
                # layernorm stats/apply for one [128, 512] half
                n = raw[:].free_size()
                mu = work.tile([2, n], bf16, tag="st", bufs=6)
                nc.scalar.activation(out=mu[:], in_=S, func=AF.Identity)
                mu2 = work.tile([2, n], f32, tag="st", bufs=6)
                nc.gpsimd.tensor_tensor(out=mu2[:], in0=mu[:], in1=mu[:],
                                        op=OP.mult)
                u = work.tile([2, n], f32, tag="st", bufs=6)
                nc.vector.scalar_tensor_tensor(
                    out=u[:], in0=Q2, scalar=1.0, in1=mu2[:],
                    op0=OP.mult, op1=OP.subtract)
                std = work.tile([2, n], f32, tag="st", bufs=6)
                nc.scalar.activation(out=std[:], in_=u[:], func=AF.Sqrt,
                                     bias=eps2[:], scale=1.0)
                r = work.tile([2, n], f32, tag="st", bufs=6)
                nc.vector.reciprocal_approx_fast(out=r[:], in_=std[:])
                r16 = work.tile([2, n], bf16, tag="st", bufs=6)
                nc.vector.tensor_copy(out=r16[:], in_=r[:])
                db = dr.tile([2, 2, n], bf16, tag="db", bufs=6)
                nc.sync.dma_start(out=db[:, 0, :], in_=mu[:])
                nc.sync.dma_start(out=db[:, 1, :], in_=r16[:])
                rb = work.tile([P, 2, n], bf16, tag="rb", bufs=4)
                dbap = db[:]
                bc_read(rb[0:64, :, :], dbap, 0, 64, [[n, 2], [1, n]])
                bc_read(rb[64:128, :, :], dbap, 2 * n, 64, [[n, 2], [1, n]])
                tmp = work.tile([P, n], bf16, tag="tmp", bufs=4)
                nc.vector.tensor_tensor(out=tmp[:], in0=raw[:],
                                        in1=rb[:, 0, :], op=OP.subtract)
                for (psl, dst) in dsts:
                    if affine:
                        tmp2 = work.tile([P, n], bf16, tag="tmp2", bufs=4)
                        nc.vector.tensor_tensor(out=tmp2[psl, :],
                                                in0=tmp[psl, :],
                                                in1=rb[psl, 1, :], op=OP.mult)
                        nc.vector.tensor_scalar(out=dst, in0=tmp2[psl, :],
                                                scalar1=gc[psl, 0:1],
                                                scalar2=bc[psl, 0:1],
                                                op0=OP.mult, op1=OP.add)
                    else:
                        nc.vector.tensor_tensor(out=dst, in0=tmp[psl, :],
                                                in1=rb[psl, 1, :], op=OP.mult)

            # q and k passes: transposed projections, pipelined stats
            pending = []
            xq_ctx = None
            xkv_sb = None
            for kind in ("q", "k"):
                tcols = TQ if kind == "q" else T
                fbase = 0 if kind == "q" else E
                if affine:
                    gc = gq_c if kind == "q" else gk_c
                    bc = bq_c if kind == "q" else bk_c
                else:
                    gc = bc = None
                if kind == "q":
                    xq_ctx = contextlib.ExitStack()
                    xqpool = xq_ctx.enter_context(
                        tc.tile_pool(name="xqpool", bufs=1))
                    xsb = xqpool.tile([P, KB, TQ], bf16, name="xq")
                    for j in range(2):
                        nc.sync.dma_start(
                            out=xsb[:, 4 * j:4 * (j + 1), :],
                            in_=xT_q[4 * j * P:4 * (j + 1) * P, :].rearrange(
                                "(kb p) t -> p kb t", p=P))
                else:
                    xsb = xkpool.tile([P, KB, T], bf16, name="xkv")
                    for j in range(4):
                        nc.sync.dma_start(
                            out=xsb[:, 2 * j:2 * (j + 1), :],
                            in_=xT_kv[2 * j * P:2 * (j + 1) * P, :].rearrange(
                                "(kb p) t -> p kb t", p=P))
                    xkv_sb = xsb
                for c in range(FC):
                    wch = wpool.tile([P, KB, P], bf16, name=f"w_{kind}{c}",
                                     tag="wqk", bufs=3)
                    nc.sync.dma_start(
                        out=wch[:],
                        in_=Wqkv[:, fbase + c * P:fbase + (c + 1) * P]
                        .rearrange("(kb p) f -> p kb f", p=P))
                    nnk = tcols // 512
                    pqs = [ps.tile([P, 512], f32, name=f"p_{kind}{c}_{nk}",
                                   tag="pqk", bufs=6) for nk in range(nnk)]
                    for kb in range(KB):
                        for nk in range(nnk):
                            nc.tensor.matmul(pqs[nk][:], wch[:, kb, :],
                                             xsb[:, kb,
                                                 nk * 512:(nk + 1) * 512],
                                             start=(kb == 0), stop=(kb == KB - 1))
                    for nk in range(nnk):
                        nsl = slice(nk * 512, (nk + 1) * 512)
                        pq = pqs[nk]
                        raw = work.tile([P, 512], bf16, tag="raw", bufs=3)
                        bias_col = bcol[:, (0 if kind == "q" else FC) + c:
                                        (0 if kind == "q" else FC) + c + 1]
                        nc.scalar.activation(out=raw[:], in_=pq[:],
                                             func=AF.Identity, bias=bias_col)
                        qsq = work.tile([P, 512], bf16, tag="qsq", bufs=3)
                        nc.vector.tensor_tensor(out=qsq[:], in0=raw[:],
                                                in1=raw[:], op=OP.mult)
                        st_ps = ps.tile([34, 512], f32, tag="stps", bufs=2)
                        nc.tensor.matmul(st_ps[0:2, :], ones_bd[:], raw[:],
                                         start=True, stop=True)
                        nc.tensor.matmul(st_ps[32:34, :], ones_bd[:], qsq[:],
                                         start=True, stop=True)
                        if kind == "q":
                            dsts = [(slice(0, 64), qhat[0:64, 0, c, nsl]),
                                    (slice(64, 128), qhat[64:128, 1, c, nsl])]
                        else:
                            dsts = [(slice(0, 128), khat[:, c, nsl])]
                        # run the non-PE tail of the previous half now
                        if pending:
                            ln_post(*pending.pop(0))
                        pending.append((raw, st_ps[0:2, :], st_ps[32:34, :],
                                        dsts, gc, bc))
                if kind == "q":
                    while pending:
                        ln_post(*pending.pop(0))
                    xq_ctx.close()
            while pending:
                ln_post(*pending.pop(0))

            # v pass: natural layout
            for c2 in range(2):
                wv = wpool.tile([P, KB, 512], bf16, name=f"w_v{c2}",
                                tag="wv", bufs=1)
                nc.sync.dma_start(
                    out=wv[:],
                    in_=Wqkv[:, 2 * E + c2 * 512:2 * E + (c2 + 1) * 512]
                    .rearrange("(kb p) f -> p kb f", p=P))
                for m in range(MKV):
                    pv = ps.tile([P, 512], f32, name=f"p_v{c2}_{m}",
                                 tag="pqk", bufs=6)
                    msl = slice(m * P, (m + 1) * P)
                    for kb in range(KB):
                        nc.tensor.matmul(pv[:], xkv_sb[:, kb, msl],
                                         wv[:, kb, :],
                                         start=(kb == 0), stop=(kb == KB - 1))
                    pv3 = pv[:].rearrange("p (h d) -> p h d", h=8)
                    hsl = slice(c2 * 8, (c2 + 1) * 8)
                    nc.scalar.activation(out=va[:, m, hsl, 0:D],
                                         in_=pv3[:], func=AF.Identity)

        late = top.enter_context(tc.tile_pool(name="late", bufs=1))
        wp_all = late.tile([P, KB, E], bf16)
        nc.sync.dma_start(out=wp_all[:],
                          in_=Wproj[:].rearrange("(kb p) f -> p kb f", p=P))

        if DEBUG:
            nc.sync.dma_start(out=dbg_q[:], in_=qhat[:])
            nc.sync.dma_start(out=dbg_k[:], in_=khat[:])
            nc.sync.dma_start(out=dbg_va[:], in_=va[:])

        # ---- phase C: attention, head pairs share the khat stationary ----
        with contextlib.ExitStack() as pc:
            cwork = pc.enter_context(tc.tile_pool(name="cwork", bufs=1))
            psc = pc.enter_context(tc.tile_pool(name="psC", bufs=1, space="PSUM"))
            LAG = 2
            for ch in range(FC):
                ys = [psc.tile([D + 1, TQ], f32, name=f"y_{ch}_{hh}",
                               tag="y", bufs=2) for hh in range(2)]
                pas = {}
                for tk in range(MKV + LAG):
                    if tk < MKV:
                        ksl = slice(tk * P, (tk + 1) * P)
                        ss = []
                        pp2 = []
                        for hh in range(2):
                            ss.append(psc.tile([P, TQ], f32,
                                               name=f"s_{ch}_{tk}_{hh}",
                                               tag="s", bufs=2))
                            pp2.append(cwork.tile([P, TQ], bf16,
                                                  name=f"pa_{ch}_{tk}_{hh}",
                                                  tag="pa", bufs=6))
                        for hh in range(2):
                            for nk in range(2):
                                nsl = slice(nk * 512, (nk + 1) * 512)
                                nc.tensor.matmul(
                                    ss[hh][:, nsl], khat[:, ch, ksl],
                                    qhat[:, hh, ch, nsl],
                                    start=True, stop=True)
                        for hh in range(2):
                            nc.scalar.activation(out=pp2[hh][:], in_=ss[hh][:],
                                                 func=AF.Exp, scale=SCALE)
                        pas[tk] = pp2
                    if tk >= LAG:
                        ppc = pas.pop(tk - LAG)
                        st = (tk - LAG == 0)
                        sp = (tk - LAG == MKV - 1)
                        for hh in range(2):
                            h = 2 * ch + hh
                            for nk in range(2):
                                nsl = slice(nk * 512, (nk + 1) * 512)
                                nc.tensor.matmul(ys[hh][:, nsl],
                                                 va[:, tk - LAG, h, :],
                                                 ppc[hh][:, nsl],
                                                 start=st, stop=sp)
                for hh in range(2):
                    r0 = hh * 64
                    y = ys[hh]
                    zrow = cwork.tile([1, TQ], f32, tag="zrow", bufs=2)
                    nc.vector.tensor_copy(out=zrow[:], in_=y[D:D + 1, :])
                    rz = cwork.tile([1, TQ], f32, tag="rz", bufs=2)
                    nc.vector.reciprocal_approx_fast(out=rz[:], in_=zrow[:])
                    zb = dr.tile([TQ], f32, tag="zb", bufs=2)
                    nc.sync.dma_start(out=zb[:], in_=rz[:])
                    rzb = cwork.tile([64, TQ], f32, tag="rzb", bufs=2)
                    bc_read(rzb[:], zb[:], 0, 64, [[1, TQ]])
                    nc.vector.tensor_tensor(out=yT[r0:r0 + 64, ch, :],
                                            in0=y[0:D, :], in1=rzb[:],
                                            op=OP.mult)

        if DEBUG:
            nc.sync.dma_start(out=dbg_y[:], in_=yT[:])

        # ---- phase D: output projection ----
        with contextlib.ExitStack() as pd:
            dwork = pd.enter_context(tc.tile_pool(name="dwork", bufs=1))
            psd = pd.enter_context(tc.tile_pool(name="psD", bufs=1, space="PSUM"))
            for m in range(TQ // P):
                msl = slice(m * P, (m + 1) * P)
                pos = [psd.tile([P, 512], f32, name=f"po_{m}_{nk}",
                                tag="po", bufs=4) for nk in range(2)]
                for kb in range(KB):
                    for nk in range(2):
                        nc.tensor.matmul(pos[nk][:], yT[:, kb, msl],
                                         wp_all[:, kb,
                                                nk * 512:(nk + 1) * 512],
                                         start=(kb == 0), stop=False)
                for nk in range(2):
                    nsl = slice(nk * 512, (nk + 1) * 512)
                    po = pos[nk]
                    nc.tensor.matmul(po[:], ones1[:], beff_row[:, nsl],
                                     start=False, stop=True)
                    osb = dwork.tile([P, 512], f32, tag="osb", bufs=4)
                    nc.scalar.activation(out=osb[:], in_=po[:],
                                         func=AF.Identity)
                    nc.sync.dma_start(out=out[msl, nsl], in_=osb[:])

    nc.finalize()
    return nc


def _get_nc(affine: bool):
    key = bool(affine)
    if key not in _BUILT:
        _BUILT[key] = _build_real(key)
    return _BUILT[key]


def kernel(x, Wqkv, bqkv, q_gamma, q_beta, k_gamma, k_beta, Wproj, bproj):
    from concourse.bass_utils import run_bass_kernel_spmd

    x = np.asarray(x, dtype=np.float32)
    Wqkv = np.asarray(Wqkv, dtype=np.float32)
    bqkv = np.asarray(bqkv, dtype=np.float32)
    Wproj = np.asarray(Wproj, dtype=np.float32)
    bproj = np.asarray(bproj, dtype=np.float32)
    q_gamma = np.asarray(q_gamma, dtype=np.float32)
    q_beta = np.asarray(q_beta, dtype=np.float32)
    k_gamma = np.asarray(k_gamma, dtype=np.float32)
    k_beta = np.asarray(k_beta, dtype=np.float32)

    affine = not (np.all(q_gamma == 1.0) and np.all(q_beta == 0.0)
                  and np.all(k_gamma == 1.0) and np.all(k_beta == 0.0))
    nc = _get_nc(affine)

    bf = ml_dtypes.bfloat16
    Wqkv_b = np.ascontiguousarray(Wqkv.astype(bf))
    Wproj_b = np.ascontiguousarray(Wproj.astype(bf))
    beff = (bqkv[2 * E:].astype(np.float64) @ Wproj.astype(np.float64)
            + bproj.astype(np.float64)).astype(np.float32).astype(bf)

    in_maps = []
    for c in range(NCORES):
        b, half = divmod(c, 2)
        xT_kv = np.ascontiguousarray(x[b].T.astype(bf))
        xT_q = np.ascontiguousarray(x[b, half * TQ:(half + 1) * TQ].T.astype(bf))
        m = {
            "xT_q": xT_q, "xT_kv": xT_kv,
            "Wqkv": Wqkv_b, "bqkv": bqkv, "beff": beff,
            "Wproj": Wproj_b,
        }
        if affine:
            m.update({"q_gamma": q_gamma, "q_beta": q_beta,
                      "k_gamma": k_gamma, "k_beta": k_beta})
        in_maps.append(m)

    global _last_in_maps
    _last_in_maps = in_maps
    res = run_bass_kernel_spmd(nc, in_maps, core_ids=list(range(NCORES)))
    y = np.empty((B, T, E), dtype=np.float32)
    for c in range(NCORES):
        b, half = divmod(c, 2)
        y[b, half * TQ:(half + 1) * TQ, :] = res.results[c]["out"]
    return y
